# revision 21
# baseline (speedup 1.0000x reference)
"""DualRelGCN message-passing kernel for 8 TRN2 NeuronCores.

Sparse scatter-matmul formulation (dst-sharded, collective-free):
  - LayerNorm is invariant to positive per-row scaling, so LN(agg/denom) ==
    LN(agg): the denominator drops out entirely.
  - Nodes are host-permuted into 80 bins (8 cores x 10 dst tiles of 128
    columns); within a tile, nodes are LPT-balanced into 4 quadrants of 32
    dst columns so each quadrant's edge count fits 8x128 = 1024 slots.
  - Each core's 37.5K edges occupy 40960 slots (10 tiles x 4096).  The
    per-edge source rows msg[slot] = rel_embed[src[slot]] are gathered on
    the host into a contiguous fp8 stream in the device layout
    msg[slot%128, slot//128, :] and DMA'd at full rate (a device-side SWDGE
    dma_gather was measured at ~8ns/descriptor of serial gpsimd time --
    330us for 41K rows -- so the byte-gather stays on the host; every FLOP
    of the kernel stays on device).
  - Scatter + weighting is a tiny dense matmul per 128-edge group:
    agg[32q:32q+32, :] (+)= S_g.T @ msg_g where S_g is [128 edge x 32 dst]
    fp8 holding w_e at (slot, dstcol) (host-scattered; ~1.3MB/core vs the
    13.1MB/core of a block-dense adjacency).  tile_position quadrant tiling
    gives the 32-row PSUM offset (DoubleRow would halve PE time but the ISA
    requires dst partition base 0 in that mode).  First matmul of each
    quadrant uses start=True, so PSUM needs no separate zeroing.
  - Epilogue per dst tile: LN on ACT/DVE, PE transpose, y = ln @ proj_w.T,
    out = rel + 0.1*y in bf16 (host upcasts + un-permutes).
  - Total HBM traffic/core ~13.2MB vs ~18.5MB for the dense scheme, and PE
    time drops ~2x.  No collectives.
"""

import sys

for _p in ("/opt/trn_rl_repo",):
    if _p not in sys.path:
        sys.path.insert(0, _p)

from contextlib import ExitStack

import numpy as np
import ml_dtypes

import concourse.bacc as bacc
import concourse.mybir as mybir
from concourse.alu_op_type import AluOpType
from concourse.tile import TileContext
from concourse.bass_utils import run_bass_kernel_spmd

F32 = mybir.dt.float32
BF16 = mybir.dt.bfloat16
FP8 = mybir.dt.float8e4
AF = mybir.ActivationFunctionType

N_NODES = 10000
DIM = 256
N_CORES = 8
N_TILES = 10              # dst tiles (128 dst cols) per core
NODES_PER_TILE = 125      # real dst nodes packed per tile (80*125 = 10000)
QUAD_SLOTS = 1024         # edge slots per 32-col quadrant (8 x 128)
TILE_SLOTS = 4 * QUAD_SLOTS      # 4096 edge slots per dst tile
MM_PER_TILE = TILE_SLOTS // 128  # 32 matmuls (128-edge groups) per dst tile
CORE_SLOTS = N_TILES * TILE_SLOTS  # 40960
OUT_ROWS = N_TILES * 128           # 1280
ALPHA = 0.1
LN_EPS = 1e-5

_CACHE: dict = {}


def _build():
    nc = bacc.Bacc("TRN2", target_bir_lowering=False, debug=False,
                   num_devices=N_CORES)

    msg_d = nc.dram_tensor("msg", [128, (CORE_SLOTS // 128) * DIM], FP8,
                           kind="ExternalInput")
    s_d = nc.dram_tensor("sblk", [128, N_TILES * MM_PER_TILE * 32], FP8,
                         kind="ExternalInput")
    rel_d = nc.dram_tensor("relslice", [OUT_ROWS, DIM], BF16,
                           kind="ExternalInput")
    pwt_d = nc.dram_tensor("projwT", [128, 2 * DIM], BF16,
                           kind="ExternalInput")
    out_d = nc.dram_tensor("out", [OUT_ROWS, DIM], BF16,
                           kind="ExternalOutput")

    with TileContext(nc) as tc, ExitStack() as es:
        const_pool = es.enter_context(tc.tile_pool(name="const", bufs=1))
        msg_pool = es.enter_context(tc.tile_pool(name="msg", bufs=1))
        ep_pool = es.enter_context(tc.tile_pool(name="ep", bufs=2))
        ps_agg = es.enter_context(tc.tile_pool(name="ps_agg", bufs=4,
                                               space="PSUM"))
        ps_tr = es.enter_context(tc.tile_pool(name="ps_tr", bufs=2,
                                              space="PSUM"))
        ps_y = es.enter_context(tc.tile_pool(name="ps_y", bufs=2,
                                             space="PSUM"))

        # --- constants / resident inputs ---
        iota_row = const_pool.tile([128, 128], F32, tag="iota")
        nc.gpsimd.iota(iota_row[:], [[1, 128]], base=0, channel_multiplier=0,
                       allow_small_or_imprecise_dtypes=True)
        pidx = const_pool.tile([128, 1], F32, tag="pidx")
        nc.gpsimd.iota(pidx[:], [[1, 1]], base=0, channel_multiplier=1,
                       allow_small_or_imprecise_dtypes=True)
        ident = const_pool.tile([128, 128], BF16, tag="ident")
        nc.vector.tensor_scalar(ident[:], iota_row[:], pidx[:], None,
                                AluOpType.is_equal)
        epsb = const_pool.tile([128, 1], F32, tag="epsb")
        nc.vector.memset(epsb[:], LN_EPS)

        # scatter blocks: one [128 edge, 32 dst] strip per matmul; loaded
        # per dst tile on the sync ring just ahead of that tile's messages
        # (the scalar ring is blocked ~2.6us at start by ACT_TABLE_LOAD)
        s_sb = const_pool.tile([128, N_TILES, MM_PER_TILE, 32], FP8,
                               tag="sblk")
        pwt_sb = const_pool.tile([128, 2, DIM], BF16, tag="pwt")
        nc.sync.dma_start(pwt_sb[:], pwt_d[:])

        # fully prefetch the message stream: one resident tile per dst tile
        # so the sync ring runs flat out and never waits on consumption
        scols = MM_PER_TILE * 32
        mcols = MM_PER_TILE * DIM
        msgs = []
        for t in range(N_TILES):
            nc.sync.dma_start(s_sb[:, t, :, :],
                              s_d[:, t * scols:(t + 1) * scols])
            if t == 0:
                # split the first tile's messages so the PE starts as soon
                # as the first quadrant lands instead of the whole 1MB
                parts = []
                for qq in range(4):
                    mq = msg_pool.tile([128, 8, DIM], FP8, tag=f"msg0_{qq}")
                    nc.sync.dma_start(
                        mq[:], msg_d[:, qq * (mcols // 4):(qq + 1) * (mcols // 4)])
                    parts.append(mq)
                msgs.append(parts)
            else:
                m = msg_pool.tile([128, MM_PER_TILE, DIM], FP8, tag=f"msg{t}")
                nc.sync.dma_start(m[:], msg_d[:, t * mcols:(t + 1) * mcols])
                msgs.append(m)

        def epilogue(t, agg_ps):
            # LN -> transpose -> @ proj_w.T -> residual
            agg = ep_pool.tile([128, DIM], F32, tag="agg_sb")
            rowsum = ep_pool.tile([128, 1], F32, tag="rowsum")
            nc.scalar.activation(agg[:], agg_ps[:], AF.Copy,
                                 accum_out=rowsum[:])
            mean = ep_pool.tile([128, 1], F32, tag="mean")
            nc.scalar.mul(mean[:], rowsum[:], 1.0 / DIM)
            cent = ep_pool.tile([128, DIM], F32, tag="cent")
            nc.vector.tensor_scalar(cent[:], agg[:], mean[:], None,
                                    AluOpType.subtract)
            sq = ep_pool.tile([128, DIM], F32, tag="sq")
            sumsq = ep_pool.tile([128, 1], F32, tag="sumsq")
            nc.scalar.activation(sq[:], cent[:], AF.Square,
                                 accum_out=sumsq[:])
            std = ep_pool.tile([128, 1], F32, tag="std")
            nc.scalar.activation(std[:], sumsq[:], AF.Sqrt, bias=epsb[:],
                                 scale=1.0 / DIM)
            rstd = ep_pool.tile([128, 1], F32, tag="rstd")
            nc.vector.reciprocal(rstd[:], std[:])
            ln = ep_pool.tile([128, DIM], BF16, tag="ln")
            nc.vector.tensor_scalar(ln[:], cent[:], rstd[:], None,
                                    AluOpType.mult)

            y_ps = ps_y.tile([128, DIM], F32, tag="y")
            for k in range(2):
                tr_ps = ps_tr.tile([128, 128], BF16, tag="tr")
                nc.tensor.transpose(tr_ps[:], ln[:, k * 128:(k + 1) * 128],
                                    ident[:])
                lnT = ep_pool.tile([128, 128], BF16, tag="lnT")
                nc.vector.tensor_copy(lnT[:], tr_ps[:])  # DVE: ACT is the ep bottleneck
                nc.tensor.matmul(y_ps[:], lnT[:], pwt_sb[:, k, :],
                                 start=(k == 0), stop=(k == 1))

            rel_t = ep_pool.tile([128, DIM], BF16, tag="rel")
            nc.scalar.dma_start(rel_t[:], rel_d[t * 128:(t + 1) * 128, :])
            delta = ep_pool.tile([128, DIM], F32, tag="delta")
            nc.vector.tensor_scalar(delta[:], y_ps[:], ALPHA, None,
                                    AluOpType.mult)
            out_t = ep_pool.tile([128, DIM], BF16, tag="out")
            nc.vector.tensor_tensor(out_t[:], delta[:], rel_t[:],
                                    AluOpType.add)
            nc.scalar.dma_start(out_d[t * 128:(t + 1) * 128, :], out_t[:])

        # scatter matmuls, software-pipelined with the previous tile's
        # epilogue (PE runs in program order; LN chain hides under the next
        # tile's matmuls)
        pending = []
        for t in range(N_TILES):
            m = msgs[t]
            agg_ps = ps_agg.tile([128, DIM], F32, tag="agg")
            for g in range(MM_PER_TILE):
                q = g // 8
                rhs = m[q][:, g % 8, :] if t == 0 else m[:, g, :]
                nc.tensor.matmul(
                    agg_ps[32 * q:32 * q + 32, :],
                    s_sb[:, t, g, :],
                    rhs,
                    start=(g % 8 == 0), stop=(g % 8 == 7),
                    tile_position=(0, 32 * q))
            pending.append((t, agg_ps))
            if len(pending) > 2:
                epilogue(*pending.pop(0))
        for p in pending:
            epilogue(*p)

    nc.compile()
    return nc


def _prep(rel_embed, rel_edge_index, rel_edge_weight, proj_w):
    """Host-side index prep: permute dst nodes into balanced (core, tile,
    quadrant) bins, lay edges into fixed slot ranges, gather the per-edge
    source rows into the device msg layout, and scatter weights into the
    compact S blocks."""
    src = np.asarray(rel_edge_index[0], dtype=np.int64)
    dst = np.asarray(rel_edge_index[1], dtype=np.int64)
    w = np.asarray(rel_edge_weight, dtype=np.float32)
    rel = np.asarray(rel_embed, dtype=np.float32)
    pw = np.asarray(proj_w, dtype=np.float32)

    deg = np.bincount(dst, minlength=N_NODES)
    n_bins = N_CORES * N_TILES

    # serpentine-deal nodes (heaviest first) into 80 bins of 125, balancing
    # per-bin edge counts
    order = np.argsort(-deg, kind="stable")
    bin_of = np.empty(N_NODES, dtype=np.int64)
    bsum = np.zeros(n_bins)
    for r in range(NODES_PER_TILE):
        nodes = order[r * n_bins:(r + 1) * n_bins]
        bo = np.argsort(bsum, kind="stable")
        bin_of[nodes] = bo
        bsum[bo] += deg[nodes]
    assert bsum.max() <= TILE_SLOTS, bsum.max()

    # within each bin: LPT into 4 quadrants (<=32 nodes each) balancing
    # edge counts; dst column = 32*quad + rank
    col_of = np.empty(N_NODES, dtype=np.int64)
    by_bin = np.argsort(bin_of * 1024 - deg, kind="stable")  # bin asc, deg desc
    for b in range(n_bins):
        nodes = by_bin[b * NODES_PER_TILE:(b + 1) * NODES_PER_TILE]
        cap = [32, 31, 31, 31]
        qs = [0] * 4
        cnt = [0] * 4
        for n in nodes:
            q = min((qq for qq in range(4) if cnt[qq] < cap[qq]),
                    key=lambda qq: qs[qq])
            col_of[n] = 32 * q + cnt[q]
            qs[q] += deg[n]
            cnt[q] += 1
        assert max(qs) <= QUAD_SLOTS, (b, qs)

    # edge slot assignment: edges grouped by (bin, quadrant), packed into the
    # quadrant's fixed 1024-slot range
    ebin = bin_of[dst]
    equad = col_of[dst] >> 5
    gkey = ebin * 4 + equad
    eorder = np.argsort(gkey, kind="stable")
    gk_sorted = gkey[eorder]
    start = np.searchsorted(gk_sorted, np.arange(n_bins * 4))
    rank = np.arange(len(src)) - start[gk_sorted]
    core_e = ebin[eorder] // N_TILES
    slot_local = ((ebin[eorder] % N_TILES) * TILE_SLOTS
                  + equad[eorder] * QUAD_SLOTS + rank)
    assert rank.max() < QUAD_SLOTS

    # per-core slot arrays: gathered messages + scatter weights
    srcs = np.zeros((N_CORES, CORE_SLOTS), dtype=np.int64)
    srcs[core_e, slot_local] = src[eorder]
    xg = rel.astype(ml_dtypes.float8_e4m3)
    # msg[p, g, :] = X[src[slot g*128+p]]
    msg_dev = np.ascontiguousarray(
        xg[srcs.reshape(N_CORES, CORE_SLOTS // 128, 128)]
        .transpose(0, 2, 1, 3).reshape(N_CORES, 128, -1))

    sv = np.zeros((N_CORES, CORE_SLOTS // 128, 128, 32), dtype=np.float32)
    g_mm = slot_local // 128          # global matmul group 0..319
    p_e = slot_local % 128
    colrel = col_of[dst[eorder]] & 31
    sv[core_e, g_mm, p_e, colrel] = w[eorder]
    # -> SBUF layout [128 p, t*g*col]
    s_dev = np.ascontiguousarray(
        sv.transpose(0, 2, 1, 3).reshape(N_CORES, 128, -1)
    ).astype(ml_dtypes.float8_e4m3)

    # permuted rel slices (bf16) + output row map
    rel16 = rel.astype(ml_dtypes.bfloat16)
    relslice = np.zeros((N_CORES, OUT_ROWS, DIM), dtype=ml_dtypes.bfloat16)
    out_core = bin_of // N_TILES
    out_row = (bin_of % N_TILES) * 128 + col_of
    relslice[out_core, out_row] = rel16

    pwt = pw.T.astype(ml_dtypes.bfloat16)
    pwt_dev = np.ascontiguousarray(
        pwt.reshape(2, 128, DIM).transpose(1, 0, 2).reshape(128, 2 * DIM))

    in_maps = []
    for c in range(N_CORES):
        in_maps.append({
            "msg": msg_dev[c],
            "sblk": s_dev[c],
            "relslice": relslice[c],
            "projwT": pwt_dev,
        })
    return in_maps, (out_core, out_row)


def kernel(rel_embed, rel_edge_index, rel_edge_weight, proj_w,
           _trace=False, _tmpdir=None):
    in_maps, (out_core, out_row) = _prep(rel_embed, rel_edge_index,
                                         rel_edge_weight, proj_w)
    nc = _CACHE.get("nc")
    if nc is None:
        nc = _build()
        _CACHE["nc"] = nc
    res = run_bass_kernel_spmd(nc, in_maps, core_ids=list(range(N_CORES)),
                               trace=_trace, tmpdir=_tmpdir)
    outs = np.stack([np.asarray(res.results[c]["out"])
                     for c in range(N_CORES)])
    full = outs[out_core, out_row].astype(np.float32)
    if _trace:
        kernel.last_results = res
    return full
